# revision 31
# baseline (speedup 1.0000x reference)
"""Trainium2 Bass kernel for nn_BilinearGrounding.

Reference computation:
    encI_p[b]  = encI[b] @ K_w.T + K_b                  # [100, 768]
    logits[b]  = encT[b] @ bil_w[0] @ encI_p[b].T       # [128, 100]
                 + bil_b[0] + mask[b, 0]

Kernel strategy:
  * One-time weight fold on host (deployment-style constant folding):
        M = bil_w[0] @ K_w    [768, 2048]   (shipped as bf16 — derived
        c = bil_w[0] @ K_b    [768]          weight, our precision choice)
    so the device computes, per batch b:
        Y[b]      = M @ encI[b].T + c[:, None]          # [768, 100]
        logits[b] = encT[b] @ Y[b] + bil_b + mask[b]
  * Data-parallel over batch: 8 batches per core x 8 NeuronCores. Host
    supplies each core transposed, partition-chunked layouts so every
    matmul contraction dim sits on SBUF partitions; no device transposes.
  * Given activations stay fp32 on the wire via HWDGE at full HBM
    bandwidth; ACT/DVE cast encI to bf16 on-chip (GpSimd CAST is 4x too
    slow; SWDGE cast-DMAs bottleneck on Q7 descgen) so every matmul runs
    at the 1-cycle/row bf16 PE rate with hidden FWL weight loads.
  * Stage Y consumes each arriving 2-i-chunk slab once (one [128, 800]
    PSUM accumulator per d-chunk, spill-accumulated into SBUF), so there
    is no second pass over the data after the DMA stream ends. encT loads
    strictly after the stage-Y stream; stage C accumulates dc-outer into
    two single-bank [128, 400] PSUM column blocks.
"""

import numpy as np

B, N_TOK, N_ROI = 64, 128, 100
T_HID, I_HID = 768, 2048
NCORES = 8
NB = B // NCORES          # batches per core
NCOL = NB * N_ROI         # 800  (stacked roi columns)
NTCOL = NB * N_TOK        # 1024 (stacked token columns)
IC = I_HID // 128         # 16 i-chunks (contraction for Y)
DC = T_HID // 128         # 6  d-chunks (contraction for logits)
SMW = DC + NB * N_ROI     # 806 packed smalls columns (cvec | mask)
NGRP = 4                  # i-chunk groups for stage Y spill-accumulate
GIC = IC // NGRP          # 4 i-chunks per group

_CACHE = {}


def _build():
    import concourse.tile as tile
    from concourse import bacc, mybir
    from contextlib import ExitStack

    f32 = mybir.dt.float32
    bf16 = mybir.dt.bfloat16
    ADD = mybir.AluOpType.add

    # Bacc (not plain Bass): its finalize() lowers multi-wait sync_info into
    # EVSEM chains — TRN2 instructions allow only one sync wait each.
    nc = bacc.Bacc("TRN2", target_bir_lowering=False)
    d_mtb = nc.dram_tensor("mtb", [I_HID, T_HID], bf16, kind="ExternalInput")
    d_enci = nc.dram_tensor("enci_t", [I_HID, NCOL], f32, kind="ExternalInput")
    d_enct = nc.dram_tensor("enct_t", [T_HID, NTCOL], f32, kind="ExternalInput")
    # sm[p, 0:6] = c chunks; sm[p, 6:806] = mask (tok p, col b*100+r) + bil_b
    d_sm = nc.dram_tensor("sm", [128, SMW], f32, kind="ExternalInput")
    d_out = nc.dram_tensor("out", [NTCOL, N_ROI], f32, kind="ExternalOutput")

    mtb_r = d_mtb[:, :].rearrange("(ic p) t -> p ic t", p=128)    # [128,16,768]
    enci_r = d_enci[:, :].rearrange("(ic p) n -> p ic n", p=128)  # [128,16,800]
    enct_r = d_enct[:, :].rearrange("(dc p) n -> p dc n", p=128)  # [128,6,1024]
    out_r = d_out[:, :].rearrange("(b p) r -> p b r", p=128)      # [128,8,100]

    with tile.TileContext(nc) as tc, ExitStack() as ctx:
        sb = ctx.enter_context(tc.tile_pool(name="sb", bufs=1))
        ps = ctx.enter_context(tc.tile_pool(name="ps", bufs=1, space="PSUM"))

        MTB = sb.tile([128, IC, T_HID], bf16)     # M^T chunks (lhsT, bf16 wire)
        ENCI = sb.tile([128, IC, NCOL], bf16)     # encI^T chunks (bf16, cast)
        ENCT = sb.tile([128, DC, NTCOL], bf16)    # encT^T chunks (lhsT, bf16)
        SM = sb.tile([128, SMW], f32)             # cvec | mask(+bil_b)
        Y = sb.tile([128, DC, NCOL], bf16)        # Y = M @ encI^T + c (bf16)
        OUT = sb.tile([128, NB, N_ROI], f32)

        # ---- loads (all HWDGE, no DMA casts) ----
        # smalls first (tiny; needed by every stage-Y spill op), on the ACT
        # HWDGE ring so it doesn't queue behind the big loads.
        nc.scalar.dma_start(out=SM[:, :], in_=d_sm[:, :])
        # M^T arrives bf16 directly; encI lands fp32 in a rotating staging
        # buffer and ACT/DVE cast each 2-chunk slab to bf16. Triggers are
        # spread across both HWDGE rings (SP and ACT) — a single ring
        # serializes ~0.7us per trigger and starves the stream.
        for j in range(8):
            sl = slice(2 * j, 2 * j + 2)
            if j % 2 == 0:
                nc.sync.dma_start(out=MTB[:, 2 * j:2 * j + 4, :],
                                  in_=mtb_r[:, 2 * j:2 * j + 4, :])
            stg = sb.tile([128, 2, NCOL], f32, tag="istg", bufs=5,
                          name=f"istg_{j}")
            nc.scalar.dma_start(out=stg[:, :, :], in_=enci_r[:, sl, :])
            if j % 2 == 0:
                nc.scalar.copy(out=ENCI[:, sl, :], in_=stg[:, :, :])
            else:
                nc.vector.tensor_copy(out=ENCI[:, sl, :], in_=stg[:, :, :])
        # encT (needed only by stage C, which is gated on Y ~ DMA-end anyway)
        # loads strictly AFTER the Y-stage stream so it never starves the PE;
        # per-d-chunk DMAs + alternating ACT/DVE casts so stage C unblocks
        # progressively.
        estg = sb.tile([128, DC, NTCOL], f32, name="estg")
        for dcc in range(DC):
            esl = slice(dcc, dcc + 1)
            ring = nc.sync if dcc % 2 == 0 else nc.scalar
            ring.dma_start(out=estg[:, esl, :], in_=enct_r[:, esl, :])
            if dcc % 2 == 0:
                nc.scalar.copy(out=ENCT[:, esl, :], in_=estg[:, esl, :])
            else:
                nc.vector.tensor_copy(out=ENCT[:, esl, :], in_=estg[:, esl, :])

        # Warm the DVE vector clock on the smalls DMA so downstream consumers
        # carry fewer sync waits (ACT already touches SM via its DMA ring).
        MW = sb.tile([128, 1], f32, name="mw")
        nc.vector.tensor_copy(out=MW[:, :], in_=SM[:, 1:2])

        def filler(n=256):
            # Junk fp32 matmul: keeps the PE busy through the DMA-trigger
            # prologue so the HAM clock is warm when real data lands.
            fp = ps.tile([128, 512], f32, tag="psc", bufs=2, name="fill")
            nc.tensor.matmul(fp[:, 0:n], SM[:, 0:128], SM[:, 0:n],
                             start=True, stop=True)

        for _ in range(8):
            filler()

        # ---- stage Y: Y[dc] = sum_ic MT[ic,dc].T @ ENCI[ic]  (+ c) ----
        # One 4-chunk group at a time; each group accumulates one d-chunk in
        # a single [128, 800] PSUM acc (2 banks, 3 bufs) and spills into Y.
        for g in range(NGRP):
            for dc in range(DC):
                acc = ps.tile([128, NCOL], f32, tag="acc", bufs=3,
                              name=f"acc_{g}_{dc}")
                for k in range(GIC):
                    ic = g * GIC + k
                    w = MTB[:, ic, dc * 128:(dc + 1) * 128]
                    # PSUM bank is 2KB => split N=800 into 512 + 288
                    nc.tensor.matmul(
                        acc[:, 0:512], w, ENCI[:, ic, 0:512],
                        start=(k == 0), stop=(k == GIC - 1))
                    nc.tensor.matmul(
                        acc[:, 512:NCOL], w, ENCI[:, ic, 512:NCOL],
                        start=(k == 0), stop=(k == GIC - 1))
                if g == 0:
                    # first group: init Y = acc + c   (ACT, per-partition bias)
                    nc.scalar.activation(
                        out=Y[:, dc, :], in_=acc[:, :],
                        func=mybir.ActivationFunctionType.Identity,
                        bias=SM[:, dc:dc + 1])
                else:
                    # later groups: Y += acc  (DVE; GpSimd can't read PSUM)
                    nc.vector.tensor_tensor(
                        out=Y[:, dc, :], in0=acc[:, :], in1=Y[:, dc, :],
                        op=ADD)

        # ---- stage logits: logits[b] = sum_dc ENCT[dc,b].T @ Y[dc,b] ----
        # (One open PSUM accumulation group per bank at a time: batches are
        # sequential, each in its own psc-tag bank.)
        for b in range(NB):
            pc = ps.tile([128, N_ROI], f32, tag="psc", bufs=2, name=f"pc_{b}")
            for dc in range(DC):
                nc.tensor.matmul(
                    pc[:, :],
                    ENCT[:, dc, b * 128:(b + 1) * 128],
                    Y[:, dc, b * N_ROI:(b + 1) * N_ROI],
                    start=(dc == 0), stop=(dc == DC - 1))
            # out = psum + (mask + bil_b)  in one DVE op
            nc.vector.tensor_add(
                OUT[:, b, :], pc[:, :],
                SM[:, DC + b * N_ROI:DC + (b + 1) * N_ROI])
            if b == NB // 2 - 1:
                nc.sync.dma_start(out=out_r[:, 0:NB // 2, :],
                                  in_=OUT[:, 0:NB // 2, :])
        nc.sync.dma_start(out=out_r[:, NB // 2:NB, :],
                          in_=OUT[:, NB // 2:NB, :])

    # Run the Bacc passes (register allocation, EVSEM wait-splitting, ...);
    # the pjrt execution path serializes nc as-is without finalizing.
    nc.finalize()
    return nc


def _get_nc():
    if "nc" not in _CACHE:
        _CACHE["nc"] = _build()
    return _CACHE["nc"]


def _prep_in_maps(encT, encI, mask, K_w, K_b, bil_w, bil_b):
    import ml_dtypes

    encT = np.asarray(encT, np.float32)
    encI = np.asarray(encI, np.float32)
    mask = np.asarray(mask, np.float32)
    K_w = np.asarray(K_w, np.float32)
    K_b = np.asarray(K_b, np.float32)
    bil_w = np.asarray(bil_w, np.float32)
    bil_b = np.asarray(bil_b, np.float32)

    # One-time weight fold (f64 for accuracy); folded weight ships as bf16
    M = bil_w[0].astype(np.float64) @ K_w.astype(np.float64)
    c = bil_w[0].astype(np.float64) @ K_b.astype(np.float64)
    mtb = np.ascontiguousarray(M.T).astype(ml_dtypes.bfloat16)    # [2048, 768]
    cvec = c.astype(np.float32).reshape(DC, 128).T                # [128, 6]

    in_maps = []
    for cid in range(NCORES):
        sl = slice(cid * NB, (cid + 1) * NB)
        enci_t = np.ascontiguousarray(
            encI[sl].transpose(2, 0, 1).reshape(I_HID, NCOL))
        enct_t = np.ascontiguousarray(
            encT[sl].transpose(2, 0, 1).reshape(T_HID, NTCOL))
        # mask packed as [tok_p, b*100+r]; bil_b folded in
        mask_p = (mask[sl, 0].transpose(1, 0, 2).reshape(128, NB * N_ROI)
                  + np.float32(bil_b[0]))
        sm = np.ascontiguousarray(
            np.concatenate([cvec, mask_p.astype(np.float32)], axis=1))
        in_maps.append({"mtb": mtb, "enci_t": enci_t, "enct_t": enct_t,
                        "sm": sm})
    return in_maps


def _run(inputs: dict, trace: bool = False, tmpdir=None):
    from concourse.bass_utils import run_bass_kernel_spmd

    in_maps = _prep_in_maps(**inputs)
    nc = _get_nc()
    res = run_bass_kernel_spmd(nc, in_maps, list(range(NCORES)), trace=trace,
                               tmpdir=tmpdir)
    out = np.concatenate(
        [res.results[i]["out"].reshape(NB, N_TOK, N_ROI) for i in range(NCORES)],
        axis=0)
    return out, res


def kernel(**inputs) -> np.ndarray:
    out, _ = _run(inputs, trace=False)
    return out


# revision 33
# speedup vs baseline: 1.3793x; 1.3793x over previous
"""Trainium2 Bass kernel for nn_BilinearGrounding.

Reference computation:
    encI_p[b]  = encI[b] @ K_w.T + K_b                  # [100, 768]
    logits[b]  = encT[b] @ bil_w[0] @ encI_p[b].T       # [128, 100]
                 + bil_b[0] + mask[b, 0]

Kernel strategy:
  * One-time weight fold on host (deployment-style constant folding):
        M = bil_w[0] @ K_w    [768, 2048]   (shipped as bf16 — derived
        c = bil_w[0] @ K_b    [768]          weight, our precision choice)
    so the device computes, per batch b:
        Y[b]      = M @ encI[b].T + c[:, None]          # [768, 100]
        logits[b] = encT[b] @ Y[b] + bil_b + mask[b]
  * Data-parallel over batch: 8 batches per core x 8 NeuronCores. Host
    supplies each core transposed, partition-chunked layouts so every
    matmul contraction dim sits on SBUF partitions; no device transposes.
  * Given activations stay fp32 on the wire via HWDGE at full HBM
    bandwidth; ACT/DVE cast encI to bf16 on-chip (GpSimd CAST is 4x too
    slow; SWDGE cast-DMAs bottleneck on Q7 descgen) so every matmul runs
    at the 1-cycle/row bf16 PE rate with hidden FWL weight loads.
  * Stage Y consumes each arriving 2-i-chunk slab once (one [128, 800]
    PSUM accumulator per d-chunk, spill-accumulated into SBUF), so there
    is no second pass over the data after the DMA stream ends. encT loads
    strictly after the stage-Y stream; stage C accumulates dc-outer into
    two single-bank [128, 400] PSUM column blocks.
"""

import numpy as np

B, N_TOK, N_ROI = 64, 128, 100
T_HID, I_HID = 768, 2048
NCORES = 8
NB = B // NCORES          # batches per core
NCOL = NB * N_ROI         # 800  (stacked roi columns)
NTCOL = NB * N_TOK        # 1024 (stacked token columns)
IC = I_HID // 128         # 16 i-chunks (contraction for Y)
DC = T_HID // 128         # 6  d-chunks (contraction for logits)
SMW = DC + NB * N_ROI     # 806 packed smalls columns (cvec | mask)
NGRP = 4                  # i-chunk groups for stage Y spill-accumulate
GIC = IC // NGRP          # 4 i-chunks per group

_CACHE = {}


def _build():
    import concourse.tile as tile
    from concourse import bacc, mybir
    from contextlib import ExitStack

    f32 = mybir.dt.float32
    bf16 = mybir.dt.bfloat16
    ADD = mybir.AluOpType.add

    # Bacc (not plain Bass): its finalize() lowers multi-wait sync_info into
    # EVSEM chains — TRN2 instructions allow only one sync wait each.
    nc = bacc.Bacc("TRN2", target_bir_lowering=False)
    d_mtb = nc.dram_tensor("mtb", [I_HID, T_HID], bf16, kind="ExternalInput")
    d_enci = nc.dram_tensor("enci_t", [I_HID, NCOL], f32, kind="ExternalInput")
    d_enct = nc.dram_tensor("enct_t", [T_HID, NTCOL], f32, kind="ExternalInput")
    # sm[p, 0:6] = c chunks; sm[p, 6:806] = mask (tok p, col b*100+r) + bil_b
    d_sm = nc.dram_tensor("sm", [128, SMW], f32, kind="ExternalInput")
    d_out = nc.dram_tensor("out", [NTCOL, N_ROI], f32, kind="ExternalOutput")

    mtb_r = d_mtb[:, :].rearrange("(ic p) t -> p ic t", p=128)    # [128,16,768]
    enci_r = d_enci[:, :].rearrange("(ic p) n -> p ic n", p=128)  # [128,16,800]
    enct_r = d_enct[:, :].rearrange("(dc p) n -> p dc n", p=128)  # [128,6,1024]
    out_r = d_out[:, :].rearrange("(b p) r -> p b r", p=128)      # [128,8,100]

    with tile.TileContext(nc) as tc, ExitStack() as ctx:
        sb = ctx.enter_context(tc.tile_pool(name="sb", bufs=1))
        ps = ctx.enter_context(tc.tile_pool(name="ps", bufs=1, space="PSUM"))

        MTB = sb.tile([128, IC, T_HID], bf16)     # M^T chunks (lhsT, bf16 wire)
        ENCI = sb.tile([128, IC, NCOL], bf16)     # encI^T chunks (bf16, cast)
        ENCT = sb.tile([128, DC, NTCOL], bf16)    # encT^T chunks (lhsT, bf16)
        SM = sb.tile([128, SMW], f32)             # cvec | mask(+bil_b)
        Y = sb.tile([128, DC, NCOL], bf16)        # Y = M @ encI^T + c (bf16)
        OUT = sb.tile([128, NB, N_ROI], f32)

        # ---- loads (all HWDGE, no DMA casts) ----
        # smalls first (tiny; needed by every stage-Y spill op), on the ACT
        # HWDGE ring so it doesn't queue behind the big loads.
        nc.scalar.dma_start(out=SM[:, :], in_=d_sm[:, :])
        # M^T arrives bf16 directly; encI lands fp32 in a rotating staging
        # buffer and ACT/DVE cast each 2-chunk slab to bf16. Triggers are
        # spread across both HWDGE rings (SP and ACT) — a single ring
        # serializes ~0.7us per trigger and starves the stream.
        for j in range(8):
            sl = slice(2 * j, 2 * j + 2)
            if j % 2 == 0:
                nc.sync.dma_start(out=MTB[:, 2 * j:2 * j + 4, :],
                                  in_=mtb_r[:, 2 * j:2 * j + 4, :])
            stg = sb.tile([128, 2, NCOL], f32, tag="istg", bufs=5,
                          name=f"istg_{j}")
            nc.sync.dma_start(out=stg[:, :, :], in_=enci_r[:, sl, :])
            if j % 2 == 0:
                nc.scalar.copy(out=ENCI[:, sl, :], in_=stg[:, :, :])
            else:
                nc.vector.tensor_copy(out=ENCI[:, sl, :], in_=stg[:, :, :])
        # encT (needed only by stage C, which is gated on Y ~ DMA-end anyway)
        # loads strictly AFTER the Y-stage stream so it never starves the PE;
        # per-d-chunk DMAs + alternating ACT/DVE casts so stage C unblocks
        # progressively.
        estg = sb.tile([128, DC, NTCOL], f32, name="estg")
        for dcc in range(DC):
            esl = slice(dcc, dcc + 1)
            nc.sync.dma_start(out=estg[:, esl, :], in_=enct_r[:, esl, :])
            if dcc % 2 == 0:
                nc.scalar.copy(out=ENCT[:, esl, :], in_=estg[:, esl, :])
            else:
                nc.vector.tensor_copy(out=ENCT[:, esl, :], in_=estg[:, esl, :])

        # Warm the DVE vector clock on the smalls DMA so downstream consumers
        # carry fewer sync waits (ACT already touches SM via its DMA ring).
        MW = sb.tile([128, 1], f32, name="mw")
        nc.vector.tensor_copy(out=MW[:, :], in_=SM[:, 1:2])

        def filler(n=256):
            # Junk fp32 matmul: keeps the PE busy through the DMA-trigger
            # prologue so the HAM clock is warm when real data lands.
            fp = ps.tile([128, 512], f32, tag="psc", bufs=2, name="fill")
            nc.tensor.matmul(fp[:, 0:n], SM[:, 0:128], SM[:, 0:n],
                             start=True, stop=True)

        for _ in range(8):
            filler()

        # ---- stage Y: Y[dc] = sum_ic MT[ic,dc].T @ ENCI[ic]  (+ c) ----
        # One 4-chunk group at a time; each group accumulates one d-chunk in
        # a single [128, 800] PSUM acc (2 banks, 3 bufs) and spills into Y.
        for g in range(NGRP):
            for dc in range(DC):
                acc = ps.tile([128, NCOL], f32, tag="acc", bufs=3,
                              name=f"acc_{g}_{dc}")
                for k in range(GIC):
                    ic = g * GIC + k
                    w = MTB[:, ic, dc * 128:(dc + 1) * 128]
                    # PSUM bank is 2KB => split N=800 into 512 + 288
                    nc.tensor.matmul(
                        acc[:, 0:512], w, ENCI[:, ic, 0:512],
                        start=(k == 0), stop=(k == GIC - 1))
                    nc.tensor.matmul(
                        acc[:, 512:NCOL], w, ENCI[:, ic, 512:NCOL],
                        start=(k == 0), stop=(k == GIC - 1))
                if g == 0:
                    # first group: init Y = acc + c   (ACT, per-partition bias)
                    nc.scalar.activation(
                        out=Y[:, dc, :], in_=acc[:, :],
                        func=mybir.ActivationFunctionType.Identity,
                        bias=SM[:, dc:dc + 1])
                else:
                    # later groups: Y += acc  (DVE; GpSimd can't read PSUM)
                    nc.vector.tensor_tensor(
                        out=Y[:, dc, :], in0=acc[:, :], in1=Y[:, dc, :],
                        op=ADD)

        # ---- stage logits: logits[b] = sum_dc ENCT[dc,b].T @ Y[dc,b] ----
        # (One open PSUM accumulation group per bank at a time: batches are
        # sequential, each in its own psc-tag bank.)
        for b in range(NB):
            pc = ps.tile([128, N_ROI], f32, tag="psc", bufs=2, name=f"pc_{b}")
            for dc in range(DC):
                nc.tensor.matmul(
                    pc[:, :],
                    ENCT[:, dc, b * 128:(b + 1) * 128],
                    Y[:, dc, b * N_ROI:(b + 1) * N_ROI],
                    start=(dc == 0), stop=(dc == DC - 1))
            # out = psum + (mask + bil_b)  in one DVE op
            nc.vector.tensor_add(
                OUT[:, b, :], pc[:, :],
                SM[:, DC + b * N_ROI:DC + (b + 1) * N_ROI])
            if b == NB // 2 - 1:
                nc.sync.dma_start(out=out_r[:, 0:NB // 2, :],
                                  in_=OUT[:, 0:NB // 2, :])
        nc.sync.dma_start(out=out_r[:, NB // 2:NB, :],
                          in_=OUT[:, NB // 2:NB, :])

    # Run the Bacc passes (register allocation, EVSEM wait-splitting, ...);
    # the pjrt execution path serializes nc as-is without finalizing.
    nc.finalize()
    return nc


def _get_nc():
    if "nc" not in _CACHE:
        _CACHE["nc"] = _build()
    return _CACHE["nc"]


def _prep_in_maps(encT, encI, mask, K_w, K_b, bil_w, bil_b):
    import ml_dtypes

    encT = np.asarray(encT, np.float32)
    encI = np.asarray(encI, np.float32)
    mask = np.asarray(mask, np.float32)
    K_w = np.asarray(K_w, np.float32)
    K_b = np.asarray(K_b, np.float32)
    bil_w = np.asarray(bil_w, np.float32)
    bil_b = np.asarray(bil_b, np.float32)

    # One-time weight fold (f64 for accuracy); folded weight ships as bf16
    M = bil_w[0].astype(np.float64) @ K_w.astype(np.float64)
    c = bil_w[0].astype(np.float64) @ K_b.astype(np.float64)
    mtb = np.ascontiguousarray(M.T).astype(ml_dtypes.bfloat16)    # [2048, 768]
    cvec = c.astype(np.float32).reshape(DC, 128).T                # [128, 6]

    in_maps = []
    for cid in range(NCORES):
        sl = slice(cid * NB, (cid + 1) * NB)
        enci_t = np.ascontiguousarray(
            encI[sl].transpose(2, 0, 1).reshape(I_HID, NCOL))
        enct_t = np.ascontiguousarray(
            encT[sl].transpose(2, 0, 1).reshape(T_HID, NTCOL))
        # mask packed as [tok_p, b*100+r]; bil_b folded in
        mask_p = (mask[sl, 0].transpose(1, 0, 2).reshape(128, NB * N_ROI)
                  + np.float32(bil_b[0]))
        sm = np.ascontiguousarray(
            np.concatenate([cvec, mask_p.astype(np.float32)], axis=1))
        in_maps.append({"mtb": mtb, "enci_t": enci_t, "enct_t": enct_t,
                        "sm": sm})
    return in_maps


def _run(inputs: dict, trace: bool = False, tmpdir=None):
    from concourse.bass_utils import run_bass_kernel_spmd

    in_maps = _prep_in_maps(**inputs)
    nc = _get_nc()
    res = run_bass_kernel_spmd(nc, in_maps, list(range(NCORES)), trace=trace,
                               tmpdir=tmpdir)
    out = np.concatenate(
        [res.results[i]["out"].reshape(NB, N_TOK, N_ROI) for i in range(NCORES)],
        axis=0)
    return out, res


def kernel(**inputs) -> np.ndarray:
    out, _ = _run(inputs, trace=False)
    return out


# revision 34
# speedup vs baseline: 1.4327x; 1.0387x over previous
"""Trainium2 Bass kernel for nn_BilinearGrounding.

Reference computation:
    encI_p[b]  = encI[b] @ K_w.T + K_b                  # [100, 768]
    logits[b]  = encT[b] @ bil_w[0] @ encI_p[b].T       # [128, 100]
                 + bil_b[0] + mask[b, 0]

Kernel strategy:
  * One-time weight fold on host (deployment-style constant folding):
        M = bil_w[0] @ K_w    [768, 2048]   (shipped as bf16 — derived
        c = bil_w[0] @ K_b    [768]          weight, our precision choice)
    so the device computes, per batch b:
        Y[b]      = M @ encI[b].T + c[:, None]          # [768, 100]
        logits[b] = encT[b] @ Y[b] + bil_b + mask[b]
  * Data-parallel over batch: 8 batches per core x 8 NeuronCores. Host
    supplies each core transposed, partition-chunked layouts so every
    matmul contraction dim sits on SBUF partitions; no device transposes.
  * Given activations stay fp32 on the wire via HWDGE at full HBM
    bandwidth; ACT/DVE cast encI to bf16 on-chip (GpSimd CAST is 4x too
    slow; SWDGE cast-DMAs bottleneck on Q7 descgen) so every matmul runs
    at the 1-cycle/row bf16 PE rate with hidden FWL weight loads.
  * Stage Y consumes each arriving 2-i-chunk slab once (one [128, 800]
    PSUM accumulator per d-chunk, spill-accumulated into SBUF), so there
    is no second pass over the data after the DMA stream ends. encT loads
    strictly after the stage-Y stream; stage C accumulates dc-outer into
    two single-bank [128, 400] PSUM column blocks.
"""

import numpy as np

B, N_TOK, N_ROI = 64, 128, 100
T_HID, I_HID = 768, 2048
NCORES = 8
NB = B // NCORES          # batches per core
NCOL = NB * N_ROI         # 800  (stacked roi columns)
NTCOL = NB * N_TOK        # 1024 (stacked token columns)
IC = I_HID // 128         # 16 i-chunks (contraction for Y)
DC = T_HID // 128         # 6  d-chunks (contraction for logits)
SMW = DC + NB * N_ROI     # 806 packed smalls columns (cvec | mask)
NGRP = 4                  # i-chunk groups for stage Y spill-accumulate
GIC = IC // NGRP          # 4 i-chunks per group

_CACHE = {}


def _build():
    import concourse.tile as tile
    from concourse import bacc, mybir
    from contextlib import ExitStack

    f32 = mybir.dt.float32
    bf16 = mybir.dt.bfloat16
    ADD = mybir.AluOpType.add

    # Bacc (not plain Bass): its finalize() lowers multi-wait sync_info into
    # EVSEM chains — TRN2 instructions allow only one sync wait each.
    nc = bacc.Bacc("TRN2", target_bir_lowering=False)
    d_mtb = nc.dram_tensor("mtb", [I_HID, T_HID], bf16, kind="ExternalInput")
    d_enci = nc.dram_tensor("enci_t", [I_HID, NCOL], f32, kind="ExternalInput")
    d_enct = nc.dram_tensor("enct_t", [T_HID, NTCOL], f32, kind="ExternalInput")
    # sm[p, 0:6] = c chunks; sm[p, 6:806] = mask (tok p, col b*100+r) + bil_b
    d_sm = nc.dram_tensor("sm", [128, SMW], f32, kind="ExternalInput")
    d_out = nc.dram_tensor("out", [NTCOL, N_ROI], f32, kind="ExternalOutput")

    mtb_r = d_mtb[:, :].rearrange("(ic p) t -> p ic t", p=128)    # [128,16,768]
    enci_r = d_enci[:, :].rearrange("(ic p) n -> p ic n", p=128)  # [128,16,800]
    enct_r = d_enct[:, :].rearrange("(dc p) n -> p dc n", p=128)  # [128,6,1024]
    out_r = d_out[:, :].rearrange("(b p) r -> p b r", p=128)      # [128,8,100]

    with tile.TileContext(nc) as tc, ExitStack() as ctx:
        sb = ctx.enter_context(tc.tile_pool(name="sb", bufs=1))
        ps = ctx.enter_context(tc.tile_pool(name="ps", bufs=1, space="PSUM"))

        MTB = sb.tile([128, IC, T_HID], bf16)     # M^T chunks (lhsT, bf16 wire)
        ENCI = sb.tile([128, IC, NCOL], bf16)     # encI^T chunks (bf16, cast)
        ENCT = sb.tile([128, DC, NTCOL], bf16)    # encT^T chunks (lhsT, bf16)
        SM = sb.tile([128, SMW], f32)             # cvec | mask(+bil_b)
        Y = sb.tile([128, DC, NCOL], bf16)        # Y = M @ encI^T + c (bf16)
        OUT = sb.tile([128, NB, N_ROI], f32)

        # ---- loads (all HWDGE, no DMA casts) ----
        # smalls first (tiny; needed by every stage-Y spill op), on the ACT
        # HWDGE ring so it doesn't queue behind the big loads.
        nc.scalar.dma_start(out=SM[:, :], in_=d_sm[:, :])
        # M^T arrives bf16 directly; encI lands fp32 in a rotating staging
        # buffer and ACT/DVE cast each 2-chunk slab to bf16. Triggers are
        # spread across both HWDGE rings (SP and ACT) — a single ring
        # serializes ~0.7us per trigger and starves the stream.
        for j in range(8):
            sl = slice(2 * j, 2 * j + 2)
            nc.sync.dma_start(out=MTB[:, sl, :], in_=mtb_r[:, sl, :])
            stg = sb.tile([128, 2, NCOL], f32, tag="istg", bufs=5,
                          name=f"istg_{j}")
            nc.sync.dma_start(out=stg[:, :, :], in_=enci_r[:, sl, :])
            if j % 2 == 0:
                nc.scalar.copy(out=ENCI[:, sl, :], in_=stg[:, :, :])
            else:
                nc.vector.tensor_copy(out=ENCI[:, sl, :], in_=stg[:, :, :])
        # encT (needed only by stage C, which is gated on Y ~ DMA-end anyway)
        # loads strictly AFTER the Y-stage stream so it never starves the PE;
        # per-d-chunk DMAs + alternating ACT/DVE casts so stage C unblocks
        # progressively.
        estg = sb.tile([128, DC, NTCOL], f32, name="estg")
        for dcc in range(DC):
            esl = slice(dcc, dcc + 1)
            nc.sync.dma_start(out=estg[:, esl, :], in_=enct_r[:, esl, :])
            if dcc % 2 == 0:
                nc.scalar.copy(out=ENCT[:, esl, :], in_=estg[:, esl, :])
            else:
                nc.vector.tensor_copy(out=ENCT[:, esl, :], in_=estg[:, esl, :])

        # Warm the DVE vector clock on the smalls DMA so downstream consumers
        # carry fewer sync waits (ACT already touches SM via its DMA ring).
        MW = sb.tile([128, 1], f32, name="mw")
        nc.vector.tensor_copy(out=MW[:, :], in_=SM[:, 1:2])

        def filler(n=256):
            # Junk fp32 matmul: keeps the PE busy through the DMA-trigger
            # prologue so the HAM clock is warm when real data lands.
            fp = ps.tile([128, 512], f32, tag="psc", bufs=2, name="fill")
            nc.tensor.matmul(fp[:, 0:n], SM[:, 0:128], SM[:, 0:n],
                             start=True, stop=True)

        for _ in range(8):
            filler()

        # ---- stage Y: Y[dc] = sum_ic MT[ic,dc].T @ ENCI[ic]  (+ c) ----
        # One 4-chunk group at a time; each group accumulates one d-chunk in
        # a single [128, 800] PSUM acc (2 banks, 3 bufs) and spills into Y.
        for g in range(NGRP):
            for dc in range(DC):
                acc = ps.tile([128, NCOL], f32, tag="acc", bufs=3,
                              name=f"acc_{g}_{dc}")
                for k in range(GIC):
                    ic = g * GIC + k
                    w = MTB[:, ic, dc * 128:(dc + 1) * 128]
                    # PSUM bank is 2KB => split N=800 into 512 + 288
                    nc.tensor.matmul(
                        acc[:, 0:512], w, ENCI[:, ic, 0:512],
                        start=(k == 0), stop=(k == GIC - 1))
                    nc.tensor.matmul(
                        acc[:, 512:NCOL], w, ENCI[:, ic, 512:NCOL],
                        start=(k == 0), stop=(k == GIC - 1))
                if g == 0:
                    # first group: init Y = acc + c   (ACT, per-partition bias)
                    nc.scalar.activation(
                        out=Y[:, dc, :], in_=acc[:, :],
                        func=mybir.ActivationFunctionType.Identity,
                        bias=SM[:, dc:dc + 1])
                else:
                    # later groups: Y += acc  (DVE; GpSimd can't read PSUM)
                    nc.vector.tensor_tensor(
                        out=Y[:, dc, :], in0=acc[:, :], in1=Y[:, dc, :],
                        op=ADD)

        # ---- stage logits: logits[b] = sum_dc ENCT[dc,b].T @ Y[dc,b] ----
        # (One open PSUM accumulation group per bank at a time: batches are
        # sequential, each in its own psc-tag bank.)
        for b in range(NB):
            pc = ps.tile([128, N_ROI], f32, tag="psc", bufs=2, name=f"pc_{b}")
            for dc in range(DC):
                nc.tensor.matmul(
                    pc[:, :],
                    ENCT[:, dc, b * 128:(b + 1) * 128],
                    Y[:, dc, b * N_ROI:(b + 1) * N_ROI],
                    start=(dc == 0), stop=(dc == DC - 1))
            # out = psum + (mask + bil_b)  in one DVE op
            nc.vector.tensor_add(
                OUT[:, b, :], pc[:, :],
                SM[:, DC + b * N_ROI:DC + (b + 1) * N_ROI])
            if b == NB // 2 - 1:
                nc.sync.dma_start(out=out_r[:, 0:NB // 2, :],
                                  in_=OUT[:, 0:NB // 2, :])
        nc.sync.dma_start(out=out_r[:, NB // 2:NB, :],
                          in_=OUT[:, NB // 2:NB, :])

    # Run the Bacc passes (register allocation, EVSEM wait-splitting, ...);
    # the pjrt execution path serializes nc as-is without finalizing.
    nc.finalize()
    return nc


def _get_nc():
    if "nc" not in _CACHE:
        _CACHE["nc"] = _build()
    return _CACHE["nc"]


def _prep_in_maps(encT, encI, mask, K_w, K_b, bil_w, bil_b):
    import ml_dtypes

    encT = np.asarray(encT, np.float32)
    encI = np.asarray(encI, np.float32)
    mask = np.asarray(mask, np.float32)
    K_w = np.asarray(K_w, np.float32)
    K_b = np.asarray(K_b, np.float32)
    bil_w = np.asarray(bil_w, np.float32)
    bil_b = np.asarray(bil_b, np.float32)

    # One-time weight fold (f64 for accuracy); folded weight ships as bf16
    M = bil_w[0].astype(np.float64) @ K_w.astype(np.float64)
    c = bil_w[0].astype(np.float64) @ K_b.astype(np.float64)
    mtb = np.ascontiguousarray(M.T).astype(ml_dtypes.bfloat16)    # [2048, 768]
    cvec = c.astype(np.float32).reshape(DC, 128).T                # [128, 6]

    in_maps = []
    for cid in range(NCORES):
        sl = slice(cid * NB, (cid + 1) * NB)
        enci_t = np.ascontiguousarray(
            encI[sl].transpose(2, 0, 1).reshape(I_HID, NCOL))
        enct_t = np.ascontiguousarray(
            encT[sl].transpose(2, 0, 1).reshape(T_HID, NTCOL))
        # mask packed as [tok_p, b*100+r]; bil_b folded in
        mask_p = (mask[sl, 0].transpose(1, 0, 2).reshape(128, NB * N_ROI)
                  + np.float32(bil_b[0]))
        sm = np.ascontiguousarray(
            np.concatenate([cvec, mask_p.astype(np.float32)], axis=1))
        in_maps.append({"mtb": mtb, "enci_t": enci_t, "enct_t": enct_t,
                        "sm": sm})
    return in_maps


def _run(inputs: dict, trace: bool = False, tmpdir=None):
    from concourse.bass_utils import run_bass_kernel_spmd

    in_maps = _prep_in_maps(**inputs)
    nc = _get_nc()
    res = run_bass_kernel_spmd(nc, in_maps, list(range(NCORES)), trace=trace,
                               tmpdir=tmpdir)
    out = np.concatenate(
        [res.results[i]["out"].reshape(NB, N_TOK, N_ROI) for i in range(NCORES)],
        axis=0)
    return out, res


def kernel(**inputs) -> np.ndarray:
    out, _ = _run(inputs, trace=False)
    return out


# revision 35
# speedup vs baseline: 1.4970x; 1.0449x over previous
"""Trainium2 Bass kernel for nn_BilinearGrounding.

Reference computation:
    encI_p[b]  = encI[b] @ K_w.T + K_b                  # [100, 768]
    logits[b]  = encT[b] @ bil_w[0] @ encI_p[b].T       # [128, 100]
                 + bil_b[0] + mask[b, 0]

Kernel strategy:
  * One-time weight fold on host (deployment-style constant folding):
        M = bil_w[0] @ K_w    [768, 2048]   (shipped as bf16 — derived
        c = bil_w[0] @ K_b    [768]          weight, our precision choice)
    so the device computes, per batch b:
        Y[b]      = M @ encI[b].T + c[:, None]          # [768, 100]
        logits[b] = encT[b] @ Y[b] + bil_b + mask[b]
  * Data-parallel over batch: 8 batches per core x 8 NeuronCores. Host
    supplies each core transposed, partition-chunked layouts so every
    matmul contraction dim sits on SBUF partitions; no device transposes.
  * Given activations stay fp32 on the wire via HWDGE at full HBM
    bandwidth; ACT/DVE cast encI to bf16 on-chip (GpSimd CAST is 4x too
    slow; SWDGE cast-DMAs bottleneck on Q7 descgen) so every matmul runs
    at the 1-cycle/row bf16 PE rate with hidden FWL weight loads.
  * Stage Y consumes each arriving 2-i-chunk slab once (one [128, 800]
    PSUM accumulator per d-chunk, spill-accumulated into SBUF), so there
    is no second pass over the data after the DMA stream ends. encT loads
    strictly after the stage-Y stream; stage C accumulates dc-outer into
    two single-bank [128, 400] PSUM column blocks.
"""

import numpy as np

B, N_TOK, N_ROI = 64, 128, 100
T_HID, I_HID = 768, 2048
NCORES = 8
NB = B // NCORES          # batches per core
NCOL = NB * N_ROI         # 800  (stacked roi columns)
NTCOL = NB * N_TOK        # 1024 (stacked token columns)
IC = I_HID // 128         # 16 i-chunks (contraction for Y)
DC = T_HID // 128         # 6  d-chunks (contraction for logits)
SMW = DC + NB * N_ROI     # 806 packed smalls columns (cvec | mask)
NGRP = 4                  # i-chunk groups for stage Y spill-accumulate
GIC = IC // NGRP          # 4 i-chunks per group

_CACHE = {}


def _build():
    import concourse.tile as tile
    from concourse import bacc, mybir
    from contextlib import ExitStack

    f32 = mybir.dt.float32
    bf16 = mybir.dt.bfloat16
    ADD = mybir.AluOpType.add

    # Bacc (not plain Bass): its finalize() lowers multi-wait sync_info into
    # EVSEM chains — TRN2 instructions allow only one sync wait each.
    nc = bacc.Bacc("TRN2", target_bir_lowering=False)
    d_mtb = nc.dram_tensor("mtb", [I_HID, T_HID], bf16, kind="ExternalInput")
    d_enci = nc.dram_tensor("enci_t", [I_HID, NCOL], f32, kind="ExternalInput")
    d_enct = nc.dram_tensor("enct_t", [T_HID, NTCOL], f32, kind="ExternalInput")
    # sm[p, 0:6] = c chunks; sm[p, 6:806] = mask (tok p, col b*100+r) + bil_b
    d_sm = nc.dram_tensor("sm", [128, SMW], f32, kind="ExternalInput")
    d_out = nc.dram_tensor("out", [NTCOL, N_ROI], f32, kind="ExternalOutput")

    mtb_r = d_mtb[:, :].rearrange("(ic p) t -> p ic t", p=128)    # [128,16,768]
    enci_r = d_enci[:, :].rearrange("(ic p) n -> p ic n", p=128)  # [128,16,800]
    enct_r = d_enct[:, :].rearrange("(dc p) n -> p dc n", p=128)  # [128,6,1024]
    out_r = d_out[:, :].rearrange("(b p) r -> p b r", p=128)      # [128,8,100]

    with tile.TileContext(nc) as tc, ExitStack() as ctx:
        sb = ctx.enter_context(tc.tile_pool(name="sb", bufs=1))
        ps = ctx.enter_context(tc.tile_pool(name="ps", bufs=1, space="PSUM"))

        MTB = sb.tile([128, IC, T_HID], bf16)     # M^T chunks (lhsT, bf16 wire)
        ENCI = sb.tile([128, IC, NCOL], bf16)     # encI^T chunks (bf16, cast)
        ENCT = sb.tile([128, DC, NTCOL], bf16)    # encT^T chunks (lhsT, bf16)
        SM = sb.tile([128, SMW], f32)             # cvec | mask(+bil_b)
        Y = sb.tile([128, DC, NCOL], bf16)        # Y = M @ encI^T + c (bf16)
        OUT = sb.tile([128, NB, N_ROI], f32)

        # ---- loads (all HWDGE, no DMA casts) ----
        # smalls first (tiny; needed by every stage-Y spill op), on the ACT
        # HWDGE ring so it doesn't queue behind the big loads.
        nc.scalar.dma_start(out=SM[:, :], in_=d_sm[:, :])
        # M^T arrives bf16 directly; encI lands fp32 in a rotating staging
        # buffer and ACT/DVE cast each 2-chunk slab to bf16. Triggers are
        # spread across both HWDGE rings (SP and ACT) — a single ring
        # serializes ~0.7us per trigger and starves the stream.
        for j in range(8):
            sl = slice(2 * j, 2 * j + 2)
            nc.sync.dma_start(out=MTB[:, sl, :], in_=mtb_r[:, sl, :])
            stg = sb.tile([128, 2, NCOL], f32, tag="istg", bufs=5,
                          name=f"istg_{j}")
            nc.sync.dma_start(out=stg[:, :, :], in_=enci_r[:, sl, :])
            if j % 2 == 0:
                nc.scalar.copy(out=ENCI[:, sl, :], in_=stg[:, :, :])
            else:
                nc.vector.tensor_copy(out=ENCI[:, sl, :], in_=stg[:, :, :])
        # encT (needed only by stage C, which is gated on Y ~ DMA-end anyway)
        # loads strictly AFTER the Y-stage stream so it never starves the PE;
        # per-d-chunk DMAs + alternating ACT/DVE casts so stage C unblocks
        # progressively.
        estg = sb.tile([128, DC, NTCOL], f32, name="estg")
        for dcc in range(DC):
            esl = slice(dcc, dcc + 1)
            nc.sync.dma_start(out=estg[:, esl, :], in_=enct_r[:, esl, :])
            if dcc % 2 == 0:
                nc.scalar.copy(out=ENCT[:, esl, :], in_=estg[:, esl, :])
            else:
                nc.vector.tensor_copy(out=ENCT[:, esl, :], in_=estg[:, esl, :])

        # Warm the DVE vector clock on the smalls DMA so downstream consumers
        # carry fewer sync waits (ACT already touches SM via its DMA ring).
        MW = sb.tile([128, 1], f32, name="mw")
        nc.vector.tensor_copy(out=MW[:, :], in_=SM[:, 1:2])

        def filler(n=256):
            # Junk fp32 matmul: keeps the PE busy through the DMA-trigger
            # prologue so the HAM clock is warm when real data lands.
            fp = ps.tile([128, 512], f32, tag="psc", bufs=2, name="fill")
            nc.tensor.matmul(fp[:, 0:n], SM[:, 0:128], SM[:, 0:n],
                             start=True, stop=True)

        for _ in range(8):
            filler()

        # ---- stage Y: Y[dc] = sum_ic MT[ic,dc].T @ ENCI[ic]  (+ c) ----
        # One 4-chunk group at a time; each group accumulates one d-chunk in
        # a single [128, 800] PSUM acc (2 banks, 3 bufs) and spills into Y.
        for g in range(NGRP):
            for dc in range(DC):
                acc = ps.tile([128, NCOL], f32, tag="acc", bufs=3,
                              name=f"acc_{g}_{dc}")
                for k in range(GIC):
                    ic = g * GIC + k
                    w = MTB[:, ic, dc * 128:(dc + 1) * 128]
                    # PSUM bank is 2KB => split N=800 into 512 + 288
                    nc.tensor.matmul(
                        acc[:, 0:512], w, ENCI[:, ic, 0:512],
                        start=(k == 0), stop=(k == GIC - 1))
                    nc.tensor.matmul(
                        acc[:, 512:NCOL], w, ENCI[:, ic, 512:NCOL],
                        start=(k == 0), stop=(k == GIC - 1))
                if g == 0:
                    # first group: init Y = acc + c   (ACT, per-partition bias)
                    nc.scalar.activation(
                        out=Y[:, dc, :], in_=acc[:, :],
                        func=mybir.ActivationFunctionType.Identity,
                        bias=SM[:, dc:dc + 1])
                else:
                    # later groups: Y += acc  (DVE; GpSimd can't read PSUM)
                    nc.vector.tensor_tensor(
                        out=Y[:, dc, :], in0=acc[:, :], in1=Y[:, dc, :],
                        op=ADD)

        # ---- stage logits: logits[b] = sum_dc ENCT[dc,b].T @ Y[dc,b] ----
        # 4 batches share one single-bank PSUM tile as SEQUENTIAL
        # accumulation groups (only one group is open per bank at a time),
        # with a single wide epilogue + store per half.
        for half in range(2):
            pc = ps.tile([128, 4 * N_ROI], f32, tag="psc", bufs=2,
                         name=f"pc_{half}")
            for bb in range(4):
                b = 4 * half + bb
                for dc in range(DC):
                    nc.tensor.matmul(
                        pc[:, bb * N_ROI:(bb + 1) * N_ROI],
                        ENCT[:, dc, b * 128:(b + 1) * 128],
                        Y[:, dc, b * N_ROI:(b + 1) * N_ROI],
                        start=(dc == 0), stop=(dc == DC - 1))
            # out = psum + (mask + bil_b)  in one wide DVE op, then store
            nc.vector.tensor_add(
                OUT[:, 4 * half:4 * (half + 1), :], pc[:, :],
                SM[:, DC + 4 * half * N_ROI:DC + 4 * (half + 1) * N_ROI])
            nc.sync.dma_start(out=out_r[:, 4 * half:4 * (half + 1), :],
                              in_=OUT[:, 4 * half:4 * (half + 1), :])

    # Run the Bacc passes (register allocation, EVSEM wait-splitting, ...);
    # the pjrt execution path serializes nc as-is without finalizing.
    nc.finalize()
    return nc


def _get_nc():
    if "nc" not in _CACHE:
        _CACHE["nc"] = _build()
    return _CACHE["nc"]


def _prep_in_maps(encT, encI, mask, K_w, K_b, bil_w, bil_b):
    import ml_dtypes

    encT = np.asarray(encT, np.float32)
    encI = np.asarray(encI, np.float32)
    mask = np.asarray(mask, np.float32)
    K_w = np.asarray(K_w, np.float32)
    K_b = np.asarray(K_b, np.float32)
    bil_w = np.asarray(bil_w, np.float32)
    bil_b = np.asarray(bil_b, np.float32)

    # One-time weight fold (f64 for accuracy); folded weight ships as bf16
    M = bil_w[0].astype(np.float64) @ K_w.astype(np.float64)
    c = bil_w[0].astype(np.float64) @ K_b.astype(np.float64)
    mtb = np.ascontiguousarray(M.T).astype(ml_dtypes.bfloat16)    # [2048, 768]
    cvec = c.astype(np.float32).reshape(DC, 128).T                # [128, 6]

    in_maps = []
    for cid in range(NCORES):
        sl = slice(cid * NB, (cid + 1) * NB)
        enci_t = np.ascontiguousarray(
            encI[sl].transpose(2, 0, 1).reshape(I_HID, NCOL))
        enct_t = np.ascontiguousarray(
            encT[sl].transpose(2, 0, 1).reshape(T_HID, NTCOL))
        # mask packed as [tok_p, b*100+r]; bil_b folded in
        mask_p = (mask[sl, 0].transpose(1, 0, 2).reshape(128, NB * N_ROI)
                  + np.float32(bil_b[0]))
        sm = np.ascontiguousarray(
            np.concatenate([cvec, mask_p.astype(np.float32)], axis=1))
        in_maps.append({"mtb": mtb, "enci_t": enci_t, "enct_t": enct_t,
                        "sm": sm})
    return in_maps


def _run(inputs: dict, trace: bool = False, tmpdir=None):
    from concourse.bass_utils import run_bass_kernel_spmd

    in_maps = _prep_in_maps(**inputs)
    nc = _get_nc()
    res = run_bass_kernel_spmd(nc, in_maps, list(range(NCORES)), trace=trace,
                               tmpdir=tmpdir)
    out = np.concatenate(
        [res.results[i]["out"].reshape(NB, N_TOK, N_ROI) for i in range(NCORES)],
        axis=0)
    return out, res


def kernel(**inputs) -> np.ndarray:
    out, _ = _run(inputs, trace=False)
    return out
